# revision 30
# baseline (speedup 1.0000x reference)
"""LorentzConv2d Trainium2 kernel (v4: host-staged p-major inputs, tree-G).

Full-input contract: kernel(x=[8,56,56,64], kernels=[64,64]) -> [8,56,56,64].
Data-parallel over batch: one image per NeuronCore (8 cores).

Host stages the padded image in an *extended p-major* bf16 layout (row
29*b+t holds padded-lin pixel 128*t+b; duplicate blocks b>=128 hold
128*(t+1)+(b-128)) so the unshifted tensor AND all 12 shifted views are
single rectangular big-descriptor DMA loads available at t=0.

Per core (padded 58x58 grid, lin p = 58*gh+gw, tiles lin = 128*t + i):
  u[p,o]  = sum_c x[p,c] g_c k[o,c]  (PE bf16; col O is sx = sum_{c>=1} x_c)
  D[p,o]  = acosh(u)^2 ~= Square(Ln(2*Relu(u-1/2)+1))  (3 ACT ops; u>=14.9
            so the ln(2u) approx err ~3e-4 cancels in the S1/sqrt(Q) ratio;
            padded pixels give exactly D=0)
  G_d[p]  = <x[p], x[p+d]>_L  (DVE mul + binary-tree halving + short reduce)
  Q[l,o]  = -box3x3(D^2)[l] + 2*sum_d boxB(d)( D * shift_d(D) * G_d )[l]
  S1[l,o] = box3x3(sx * D)[l]
  out_o   = (S1/63) * rsqrt(|Q|)  (o>=1);  out_0 = sqrt(1 + sum_o out_o^2)
Box sums: banded-Toeplitz matmuls on PE (bf16, fp32 PSUM accum).
Output written p-major; host untangles. Validated 1.6e-4 vs reference.
"""

import os
import numpy as np

import concourse.bass as bass
import concourse.bacc as bacc
import concourse.tile as tile
from concourse import mybir
from concourse.bass_utils import run_bass_kernel_spmd

F32 = mybir.dt.float32
BF16 = mybir.dt.bfloat16
AF = mybir.ActivationFunctionType
OP = mybir.AluOpType

# geometry
H = W = 56
C = 64
O = 64
GH = GW = 58              # padded grid
NG = GH * GW              # 3364
NT = 27                   # pixel tiles of 128
NP = NT * 128             # 3456 compute pixels
NB_BLK = 246              # extended p-major blocks (128 + max shift 118)
TS = 29                   # t-slots per p-major block
NSLOT = 30                # SBUF field slots (1 pad front, data, 2 pad back)

DELTAS = [(0, 1), (0, 2), (1, -2), (1, -1), (1, 0), (1, 1), (1, 2),
          (2, -2), (2, -1), (2, 0), (2, 1), (2, 2)]
ND = len(DELTAS)


def _interval(d):
    return range(max(-1, -1 - d), min(1, 1 - d) + 1)


def _build_passes():
    box33 = [58 * a + b for a in (-1, 0, 1) for b in (-1, 0, 1)]
    passes = [("diag", None, -1.0, box33, "q"),
              ("s1", None, 1.0, box33, "s")]
    for di, (dh, dw) in enumerate(DELTAS):
        box = [58 * a + b for a in _interval(dh) for b in _interval(dw)]
        passes.append((f"d{di}", di, 2.0, box, "q"))
    return passes


def _build_bands(passes):
    mats = []
    sides = []
    for (_, _, coeff, box, _) in passes:
        bs = set(box)
        plist = []
        for j in (-1, 0, 1):
            T = np.zeros((128, 128), dtype=np.float32)
            for t in bs:
                d = t - 128 * j
                if -127 <= d <= 127:
                    idx = np.arange(max(0, d), 128 + min(0, d))
                    T[idx, idx - d] = coeff
            if np.any(T):
                plist.append((j, len(mats)))
                mats.append(T)
        sides.append(plist)
    return np.stack(mats), sides


PASSES = _build_passes()
BANDS, PASS_SIDES = _build_bands(PASSES)
NB = BANDS.shape[0]

UCHUNKS = [(0, 7), (7, 7), (14, 7), (21, 6)]     # u matmul psum chunks
BCHUNKS = [(0, 8), (8, 8), (16, 8), (24, 3)]     # box psum chunks (1 bank ea)

GEXP_ACT = set(range(8))          # deltas whose G-broadcast runs on ACT
T2_POOL = {0, 1, 2, 3}            # deltas whose t2 runs on gpsimd


def build_nc():
    nc = bacc.Bacc(None)
    xpe_in = nc.declare_dram_parameter("xpe", [NB_BLK * TS, C], BF16,
                                       isOutput=False)
    gxpe_in = nc.declare_dram_parameter("gxpe", [NB_BLK * TS, C], BF16,
                                        isOutput=False)
    gk_in = nc.declare_dram_parameter("gk16", [C, O + 1], BF16, isOutput=False)
    bands_in = nc.declare_dram_parameter("bands", [128, NB * 128], BF16,
                                         isOutput=False)
    id_in = nc.declare_dram_parameter("id16", [128, 128], BF16, isOutput=False)
    out_ext = nc.declare_dram_parameter("out", [NP, O], F32, isOutput=True)

    xpe = xpe_in.rearrange("(b t) c -> b t c", t=TS)
    gxpe = gxpe_in.rearrange("(b t) c -> b t c", t=TS)

    with nc.allow_low_precision("bf16 fields/reduces; validated 1.6e-4"), \
            tile.TileContext(nc) as tc:
        with (
            tc.tile_pool(name="dram", bufs=1, space="DRAM") as dpool,
            tc.tile_pool(name="sg", bufs=1) as sg,
        ):
            dpe = dpool.tile([NB_BLK * TS, C], BF16)   # d16 ext p-major
            dpew = dpe.rearrange("(b t) c -> b t c", t=TS)

            # ---- hot inputs first: x16/gx16 + all 12 shifted x views.
            # DMA packets round-robin across outstanding transfers, so the
            # first few xs loads are split into halves to land early (G can
            # start), and the cold bands load is issued after the chain.
            x16 = sg.tile([128, NT, C], BF16)
            nc.sync.dma_start(out=x16[:], in_=xpe[0:128, 0:NT, :])
            gx16 = sg.tile([128, NT, C], BF16)
            nc.gpsimd.dma_start(out=gx16[:], in_=gxpe[0:128, 0:NT, :])
            id_sb = sg.tile([128, 128], BF16)
            nc.scalar.dma_start(out=id_sb[:], in_=id_in[:])
            gk_sb = sg.tile([C, O + 1], BF16)
            nc.scalar.dma_start(out=gk_sb[:], in_=gk_in[:])
            bands_sb = sg.tile([128, NB, 128], BF16)
            bands_r = bands_in.rearrange("p (b m) -> p b m", m=128)
            nc.sync.dma_start(out=bands_sb[:, 0:6, :], in_=bands_r[:, 0:6, :])
            xs_t = []
            for di, (dh, dw) in enumerate(DELTAS):
                dlin = 58 * dh + dw
                xs = sg.tile([128, NT, C], BF16, tag=f"xs{di}", name=f"xs{di}")
                eng = nc.sync if di % 2 == 0 else nc.gpsimd
                if di < 4:
                    h = NT // 2
                    eng.dma_start(out=xs[:, 0:h, :],
                                  in_=xpe[dlin:dlin + 128, 0:h, :])
                    eng.dma_start(out=xs[:, h:NT, :],
                                  in_=xpe[dlin:dlin + 128, h:NT, :])
                else:
                    eng.dma_start(out=xs[:], in_=xpe[dlin:dlin + 128, 0:NT, :])
                xs_t.append(xs)
            cmhalf = sg.tile([128, 1], F32)
            nc.gpsimd.memset(cmhalf[:], -0.5)

            d16 = sg.tile([128, NSLOT, C], BF16)
            nc.gpsimd.memset(d16[:, NT:NSLOT, :], 0.0)
            sx16 = sg.tile([128, NT], BF16)
            g16 = [sg.tile([128, NT], BF16, tag=f"g{di}", name=f"g{di}")
                   for di in range(ND)]

            # ================= phase A: u, sx, dists (PE + ACT) ==========
            # batched per chunk: 7 PE transposes -> one big ACT copy ->
            # 7 PE matmuls -> 3-op ACT chain. No per-tile ping-pong.
            sbA = sg
            xT = sbA.tile([64, NT, 128], BF16)
            um = sbA.tile([128, NT, O], F32)
            um2 = sbA.tile([128, NT, O], F32)
            tgp = [sbA.tile([128, NT, C], BF16, tag=f"tg{i}",
                            name=f"tg{i}") for i in range(2)]
            trp = [sbA.tile([128, NT, 32], BF16, tag=f"tr{i}",
                            name=f"tr{i}") for i in range(2)]
            tgg = [sbA.tile([128, NT, C], BF16, tag=f"tgg{i}",
                            name=f"tgg{i}") for i in range(2)]
            trg = [sbA.tile([128, NT, 32], BF16, tag=f"trg{i}",
                            name=f"trg{i}") for i in range(2)]

            def g_group(di, pool=False):
                """G product+tree (DVE, or gpsimd); final reduce on DVE."""
                eng = nc.gpsimd if pool else nc.vector
                tg = (tgg if pool else tgp)[di % 2]
                tr = (trg if pool else trp)[di % 2]
                eng.tensor_mul(tg[:], xs_t[di][:], gx16[:])
                eng.tensor_add(tr[:], tg[:, :, 0:32], tg[:, :, 32:64])
                eng.tensor_add(tr[:, :, 0:16], tr[:, :, 0:16],
                               tr[:, :, 16:32])
                nc.vector.tensor_reduce(g16[di][:], tr[:, :, 0:16],
                                        axis=mybir.AxisListType.X, op=OP.add)

            with (
                tc.tile_pool(name="psA", bufs=1, space="PSUM") as psA,
                tc.tile_pool(name="psT", bufs=1, space="PSUM") as psT,
            ):
                psu_g = [psA.tile([128, 7, O + 1], F32, tag=f"psu{i}",
                                  name=f"psu{i}") for i in range(4)]
                pst_g = [psT.tile([C, 7, 128], BF16, tag=f"pst{i}",
                                  name=f"pst{i}") for i in range(4)]
                # all transposes first: PE streams without ACT round trips
                for gi, (t0, tn) in enumerate(UCHUNKS):
                    for i in range(tn):
                        nc.tensor.transpose(pst_g[gi][:, i, :],
                                            x16[:, t0 + i, :], id_sb[:])
                    nc.scalar.copy(xT[:, t0:t0 + tn, :], pst_g[gi][:, :tn, :])
                for gi, (t0, tn) in enumerate(UCHUNKS):
                    for i in range(tn):
                        nc.tensor.matmul(psu_g[gi][:, i, :],
                                         xT[:, t0 + i, :],
                                         gk_sb[:], start=True, stop=True)
                    nc.scalar.activation(um[:, t0:t0 + tn, :],
                                         psu_g[gi][:, :tn, 0:O],
                                         AF.Relu, bias=cmhalf[:])
                    nc.scalar.activation(um2[:, t0:t0 + tn, :],
                                         um[:, t0:t0 + tn, :],
                                         AF.Ln, bias=1.0, scale=2.0)
                    nc.scalar.activation(d16[:, t0:t0 + tn, :],
                                         um2[:, t0:t0 + tn, :], AF.Square)
                    nc.scalar.copy(sx16[:, t0:t0 + tn], psu_g[gi][:, :tn, O])
                # delta bands load once the ACT queue has cleared the chain
                nc.scalar.dma_start(out=bands_sb[:, 6:NB, :],
                                    in_=bands_r[:, 6:NB, :])
                # first half of the G groups (DVE) overlaps the chain
                for di in range(6):
                    g_group(di)

            # ---- d16 to DRAM ext p-major; prefetch all 12 shifted d views
            # (all on the sync queue, which is idle from here on — keeps
            # gpsimd free for its BC tensor work)
            nc.sync.dma_start(out=dpew[0:128, :, :], in_=d16[:, 0:TS, :])
            nc.sync.dma_start(out=dpew[128:NB_BLK, :, :],
                              in_=d16[0:NB_BLK - 128, 1:TS + 1, :])
            ds_t = []
            for di, (dh, dw) in enumerate(DELTAS):
                dlin = 58 * dh + dw
                ds = sg.tile([128, NT, O], BF16, tag=f"ds{di}", name=f"ds{di}")
                nc.sync.dma_start(out=ds[:],
                                  in_=dpew[dlin:dlin + 128, 0:NT, :])
                ds_t.append(ds)

            # ===== phase BC: fields + banded box matmuls =====
            with (
                tc.tile_pool(name="psQ", bufs=1, space="PSUM") as psQ,
                tc.tile_pool(name="psS", bufs=1, space="PSUM") as psS,
                tc.tile_pool(name="sbB", bufs=1) as sbB,
            ):
                ps_q = psQ.tile([128, NT, O], F32)
                ps_s = psS.tile([128, NT, O], F32)

                NF = 4
                fbuf = [sbB.tile([128, NSLOT, O], BF16, tag=f"f{i}",
                                 name=f"f{i}") for i in range(NF)]
                fdiag = sbB.tile([128, NSLOT, O], BF16)
                fs1 = sbB.tile([128, NSLOT, O], BF16)
                for f in fbuf + [fdiag, fs1]:
                    nc.gpsimd.memset(f[:, 0, :], 0.0)
                    nc.gpsimd.memset(f[:, NT + 1:NSLOT, :], 0.0)

                n_writes_q = sum(len(PASS_SIDES[pi])
                                 for pi, p in enumerate(PASSES) if p[4] == "q")
                n_writes_s = sum(len(PASS_SIDES[pi])
                                 for pi, p in enumerate(PASSES) if p[4] == "s")
                wq = [0] * len(BCHUNKS)
                ws = [0] * len(BCHUNKS)

                def box_pass(pi, fld):
                    tgt_kind = PASSES[pi][4]
                    tgt, wcnt, wtot = ((ps_q, wq, n_writes_q)
                                       if tgt_kind == "q"
                                       else (ps_s, ws, n_writes_s))
                    for (j, bi) in PASS_SIDES[pi]:
                        for ci, (c0, cw) in enumerate(BCHUNKS):
                            nc.tensor.matmul(
                                tgt[:, c0:c0 + cw, :],
                                bands_sb[:, bi, :],
                                fld[:, 1 + c0 + j:1 + c0 + j + cw, :],
                                start=(wcnt[ci] == 0),
                                stop=(wcnt[ci] == wtot - 1),
                                skip_group_check=True)
                            wcnt[ci] += 1

                # diag + s1 fields first so PE streams early
                nc.scalar.activation(fdiag[:, 1:NT + 1, :], d16[:, 0:NT, :],
                                     AF.Square)
                box_pass(0, fdiag)
                nc.gpsimd.tensor_mul(
                    fs1[:, 1:NT + 1, :], d16[:, 0:NT, :],
                    sx16[:].unsqueeze(2).to_broadcast([128, NT, O]))
                box_pass(1, fs1)

                gexp_p = [sbB.tile([128, NT, O], BF16, tag=f"ge{i}",
                                   name=f"ge{i}") for i in range(3)]
                t2p = [sbB.tile([128, NT, O], BF16, tag=f"t2{i}",
                                name=f"t2{i}") for i in range(2)]
                # all 12 G-broadcasts on ACT, ahead of their f-muls
                for di in range(ND):
                    nc.scalar.copy(
                        gexp_p[di % 3][:],
                        g16[di][:].unsqueeze(2).to_broadcast([128, NT, O]))
                    f = fbuf[di % NF]
                    t2 = t2p[di % 2]
                    t2eng = nc.gpsimd if di in T2_POOL else nc.vector
                    t2eng.tensor_mul(t2[:], ds_t[di][:], d16[:, 0:NT, :])
                    nc.vector.tensor_mul(f[:, 1:NT + 1, :], t2[:],
                                         gexp_p[di % 3][:])
                    box_pass(2 + di, f)
                    if di < 2:
                        g_group(6 + di)
                    elif di < 6:
                        g_group(6 + di, pool=True)

                # ================= phase D: normalize & emit ==============
                osb = sbB.tile([128, NT, O], F32)
                rr = sbB.tile([128, 8, O], F32)
                for ci, (c0, cw) in enumerate(BCHUNKS):
                    nc.scalar.activation(rr[:, :cw, :],
                                         ps_q[:, c0:c0 + cw, :],
                                         AF.Abs_reciprocal_sqrt)
                    nc.vector.scalar_tensor_tensor(
                        out=osb[:, c0:c0 + cw, :],
                        in0=ps_s[:, c0:c0 + cw, :],
                        scalar=1.0 / 63.0, in1=rr[:, :cw, :],
                        op0=OP.mult, op1=OP.mult)
                s2 = sbB.tile([128, NT, O - 1], BF16)
                nc.scalar.activation(s2[:], osb[:, :, 1:O], AF.Square)
                red = sbB.tile([128, NT], F32)
                nc.vector.tensor_reduce(red[:], s2[:],
                                        axis=mybir.AxisListType.X, op=OP.add)
                r0 = sbB.tile([128, NT], F32)
                nc.scalar.activation(r0[:], red[:], AF.Abs_reciprocal_sqrt,
                                     bias=1.0)
                nc.vector.scalar_tensor_tensor(
                    out=osb[:, :, 0], in0=red[:], scalar=1.0, in1=r0[:],
                    op0=OP.add, op1=OP.mult)
                nc.sync.dma_start(
                    out=out_ext.rearrange("(p t) c -> p t c", t=NT),
                    in_=osb[:])
    nc.finalize()
    return nc


_NC_CACHE = None


def _get_nc():
    global _NC_CACHE
    if _NC_CACHE is None:
        _NC_CACHE = build_nc()
    return _NC_CACHE


def host_consts(kernels):
    gk_ext = np.zeros((C, O + 1), dtype=np.float32)
    gk_ext[:, :O] = kernels.astype(np.float32).T
    gk_ext[1:, :O] *= -1.0
    gk_ext[1:, O] = 1.0
    return gk_ext


def host_pmajor(xi):
    """[56,56,64] fp32 -> extended p-major bf16 [246*29, 64]."""
    import ml_dtypes
    grid = np.zeros((GH, GW, C), dtype=np.float32)
    grid[1:57, 1:57] = xi
    lin = np.zeros((128 * (TS + 1), C), dtype=ml_dtypes.bfloat16)
    lin[:NG] = grid.reshape(-1, C).astype(ml_dtypes.bfloat16)
    blk = lin.reshape(TS + 1, 128, C)                 # [t, b, c]
    pm = np.zeros((NB_BLK, TS, C), dtype=ml_dtypes.bfloat16)
    pm[0:128] = blk[0:TS].transpose(1, 0, 2)          # block b, slot t
    pm[128:NB_BLK] = blk[1:TS + 1, 0:NB_BLK - 128].transpose(1, 0, 2)
    return np.ascontiguousarray(pm.reshape(NB_BLK * TS, C))


def kernel(x, kernels):
    import ml_dtypes
    x = np.asarray(x, dtype=np.float32)
    kernels = np.asarray(kernels, dtype=np.float32)
    B = x.shape[0]
    assert x.shape == (B, H, W, C) and B == 8, x.shape
    gk16 = np.ascontiguousarray(host_consts(kernels).astype(ml_dtypes.bfloat16))
    id16 = np.eye(128, dtype=ml_dtypes.bfloat16)
    # [NB,128,128] -> partition-major [128, NB*128]: one descriptor/partition
    bands16 = np.ascontiguousarray(
        BANDS.astype(ml_dtypes.bfloat16).transpose(1, 0, 2).reshape(128, -1))
    nc = _get_nc()
    in_maps = []
    for i in range(8):
        xpe = host_pmajor(x[i])
        gxpe = xpe.copy()
        gxpe[:, 0] = (-gxpe[:, 0].astype(np.float32)).astype(ml_dtypes.bfloat16)
        in_maps.append({
            "xpe": xpe,
            "gxpe": np.ascontiguousarray(gxpe),
            "gk16": gk16,
            "bands": bands16,
            "id16": id16,
        })
    res = run_bass_kernel_spmd(nc, in_maps, core_ids=list(range(8)),
                               trace=bool(int(os.environ.get("KTRACE", "0"))))
    if res.exec_time_ns is not None:
        print(f"HW exec time: {res.exec_time_ns} ns")
    outs = []
    for i in range(8):
        pm = res.results[i]["out"].reshape(128, NT, O)
        lin = np.ascontiguousarray(pm.transpose(1, 0, 2)).reshape(NP, O)
        outs.append(lin[:NG].reshape(GH, GW, O)[1:57, 1:57, :])
    return np.stack(outs).astype(np.float32)
